# revision 50
# baseline (speedup 1.0000x reference)
"""Trainium2 Bass kernel for nn_MixedLinearV2 (moe_routing).

y[b,s,o] = sum_i x[b,s,i] * (W[o,i]*coeff[o,i]) + b[o]*rowscale[o]
  coeff[o,i]  = sum_k weights[k] * row_mask[k,o] * col_mask[k,i]
  rowscale[o] = sum_k weights[k] * row_mask[k,o]

Strategy: data-parallel over batch (8 batch elements -> 8 NeuronCores).
coeff/rowscale depend only on the 9 mixing weights and the static
masks, so W_mix = W*coeff and b_mix = b*rowscale are folded on the host
during the (already required) shard/layout prep; each core then runs a
pure [4096,1024]x[1024,4096] GEMM with a bias-add on eviction.

Per core: x and W_mix are laid out contraction-dim-major (i on
partitions) so they DMA straight into matmul operand tiles. W_mix ships
bf16 (8MB instead of 16MB -- the startup is DMA-bound) and is upcast to
f32r on the otherwise-idle DVE before the matmuls read it. Main loop:
for each 128-row tile of x: 64 matmuls (8 k-tiles x 8 512-wide out
chunks, 4 PSUM banks active + 4 draining), DVE bias-add eviction, DMA
out. The first 8 s-tiles' half-0 passes (chunks 0-3 only) chase the W
DMA stream so the PE never idles while W arrives; the last s-tile
evicts per-chunk so the final writeback drains early.

Matmul dtype: float32r for both operands (1 col/cycle on the PE --
227ns/matmul sustained, the same rate as bf16 but with ~13-bit
mantissas). Net norm relative error ~1.4e-3, dominated by the bf16
quantization of W_mix.
"""

import sys
import types

import numpy as np

# ---- constants (hardcoded from the problem spec) ----
B, S, IN, OUT = 8, 4096, 1024, 4096
IN_DIMS = (512, 768, 1024)
OUT_MULTS = (2, 3, 4)
K9 = 9
P = 128
KT = IN // P          # 8 k-tiles
ST = S // P           # 32 s-tiles
OC = OUT // 512       # 8 out chunks of 512
N_CORES = 8

MAIN_DT_NAME = "f32r"  # one of: f32r, bf16, f32


def _ensure_ntff_hook():
    """Register the antenv.axon_hooks shim so trace=True can profile."""
    if 'antenv.axon_hooks' in sys.modules:
        return
    try:
        import antenv
    except ImportError:
        return
    mod = types.ModuleType('antenv.axon_hooks')
    mod._hook = None
    mod.set_axon_ntff_profile_hook = lambda h: setattr(mod, '_hook', h)
    mod.get_axon_ntff_profile_hook = lambda: mod._hook
    sys.modules['antenv.axon_hooks'] = mod
    antenv.axon_hooks = mod
    try:
        from trn_agent_boot.trn_boot import _ntff_profile_via_ctypes
        mod.set_axon_ntff_profile_hook(
            _ntff_profile_via_ctypes('/opt/axon/libaxon_pjrt.so'))
    except Exception:
        pass


def _mix_np(weights, W, bias):
    """Host-side mixing: W_mix = W*coeff, b_mix = b*rowscale."""
    out_dims = np.array([m * i for i in IN_DIMS for m in OUT_MULTS])
    in_dims = np.array([i for i in IN_DIMS for _ in OUT_MULTS])
    row_mask = (np.arange(OUT)[None, :] < out_dims[:, None]).astype(np.float32)
    col_mask = (np.arange(IN)[None, :] < in_dims[:, None]).astype(np.float32)
    coeff = np.einsum('k,ko,ki->oi', weights, row_mask, col_mask,
                      dtype=np.float32)
    W_mix = (W * coeff).astype(np.float32)
    b_mix = (bias * (weights @ row_mask)).astype(np.float32)
    return W_mix, b_mix


_BUILT = {}


def _build(main_dt_name=MAIN_DT_NAME):
    """Build + compile the SPMD Bass program (one program, 8 cores)."""
    if main_dt_name in _BUILT:
        return _BUILT[main_dt_name]

    import concourse.bacc as bacc
    import concourse.mybir as mybir
    from concourse.tile import TileContext

    F32 = mybir.dt.float32
    F32R = mybir.dt.float32r
    MAIN_DT = {"f32r": F32R, "bf16": mybir.dt.bfloat16, "f32": F32}[main_dt_name]
    # x/W stream in pre-cast to MAIN_DT (sync DMA cannot cast)
    IO_DT = MAIN_DT

    nc = bacc.Bacc("TRN2", target_bir_lowering=False, debug=False,
                   num_devices=N_CORES)

    BF16 = mybir.dt.bfloat16
    # xT[s, p, it, q] = x[s*128+q, it*128+p] : k-major 128-row tiles
    xT_d = nc.declare_dram_parameter("xT", [ST, P, KT, P], IO_DT, isOutput=False)
    # WT[p, it, o] = W_mix[o, it*128+p], shipped bf16 (half the startup
    # DMA bytes) and DVE-upcast to MAIN_DT before the matmuls read it
    wT_d = nc.declare_dram_parameter("WT", [P, KT, OUT], BF16, isOutput=False)
    # b_mix replicated across partitions on host
    b_d = nc.declare_dram_parameter("bmix", [P, OUT], F32, isOutput=False)
    y_d = nc.declare_dram_parameter("y", [S, OUT], F32, isOutput=True)

    with TileContext(nc) as tc:
        with (
            tc.tile_pool(name="persist", bufs=1) as persist,
            tc.tile_pool(name="xT_pool", bufs=8) as xT_pool,
            tc.tile_pool(name="ysb_pool", bufs=2) as ysb_pool,
            tc.tile_pool(name="wstg_pool", bufs=3) as wstg_pool,
            tc.tile_pool(name="ps_pool", bufs=8, space="PSUM") as ps_pool,
        ):
            wmixT = persist.tile([P, KT, OUT], MAIN_DT)   # [i_part, i_outer, o]
            bias_sb = persist.tile([P, OUT], F32)

            # PE warm-up: dummy N=512 matmuls (100% streaming duty) fill
            # the DMA-wait window at kernel start so the HAM clock gate
            # flips to 8/8 (2.4GHz) before the real matmul stream begins
            # (~3.4us of busy PE flips it).
            warm_rhs = persist.tile([P, 512], MAIN_DT)
            nc.any.memzero(warm_rhs[:])
            warm_ps = ps_pool.tile([P, 512], F32, tag="ps", name="warm_ps")
            for _ in range(14):
                nc.tensor.matmul(warm_ps[:], warm_rhs[:, 0:P], warm_rhs[:],
                                 start=True, stop=True)

            xT_tiles = {}

            def fetch_xT(s):
                if s < ST and s not in xT_tiles:
                    xT_tiles[s] = xT_pool.tile([P, KT, P], MAIN_DT, tag="xT",
                                               name=f"xT_{s}")
                    nc.sync.dma_start(xT_tiles[s][:], xT_d[s])

            def fetch_w_bf16(ocx):
                """Dispatch the bf16 DMAs for a W chunk (two half-chunks)."""
                tiles = []
                for h in range(2):
                    lo = ocx * 512 + h * 256
                    wstg = wstg_pool.tile([P, KT, 256], BF16, tag="wstg",
                                          name=f"wstg_{ocx}_{h}")
                    nc.sync.dma_start(wstg[:], wT_d[:, :, lo:lo + 256])
                    tiles.append((lo, wstg))
                return tiles

            def upcast_w(tiles, fine=False):
                if fine:
                    # per-k-slice copies, both halves interleaved, so the
                    # first matmul group waits on 2 copies instead of 9
                    for it in range(KT):
                        for lo, wstg in tiles:
                            nc.vector.tensor_copy(
                                wmixT[:, it, lo:lo + 256], wstg[:, it, :])
                else:
                    # coarse upcasts run on the otherwise-idle GpSimd so
                    # the DVE keeps its bandwidth for evictions
                    for lo, wstg in tiles:
                        nc.gpsimd.tensor_copy(wmixT[:, :, lo:lo + 256],
                                              wstg[:])

            # DMA FIFO: x0 first, W chunks right behind it (all bf16,
            # upcast on the DVE), x tiles and bias interleaved by the time
            # each is first needed.
            fetch_xT(0)
            w0 = fetch_w_bf16(0)
            w1 = fetch_w_bf16(1)
            upcast_w(w0, fine=True)
            upcast_w(w1, fine=True)
            fetch_xT(1)
            nc.sync.dma_start(bias_sb[:, :OUT // 2], b_d[:, :OUT // 2])
            w2 = fetch_w_bf16(2)
            w3 = fetch_w_bf16(3)
            upcast_w(w2)
            upcast_w(w3)
            fetch_xT(2)
            fetch_xT(3)
            nc.sync.dma_start(bias_sb[:, OUT // 2:], b_d[:, OUT // 2:])
            fetch_xT(4)
            w4 = fetch_w_bf16(4)
            fetch_xT(5)
            fetch_xT(6)
            fetch_xT(7)

            def evict(s, half, yps, fine=False):
                ysb = ysb_pool.tile([P, 2048], F32, tag="ysb",
                                    name=f"ysb_{s}_{half}")
                for j in range(4):
                    ocx = half * 4 + j
                    nc.vector.tensor_tensor(
                        ysb[:, j * 512:(j + 1) * 512], yps[j][:],
                        bias_sb[:, ocx * 512:(ocx + 1) * 512],
                        mybir.AluOpType.add)
                    if fine:
                        # per-chunk DMA so the final writeback pipelines
                        # behind each bank instead of waiting for all four
                        nc.sync.dma_start(
                            y_d[s * P:(s + 1) * P,
                                ocx * 512:(ocx + 1) * 512],
                            ysb[:, j * 512:(j + 1) * 512])
                if not fine:
                    nc.sync.dma_start(
                        y_d[s * P:(s + 1) * P,
                            half * 2048:(half + 1) * 2048],
                        ysb[:])

            def main_half(s, half, fine=False):
                yps = []
                for j in range(4):
                    ocx = half * 4 + j
                    yp = ps_pool.tile([P, 512], F32, tag="ps",
                                      name=f"yps_{s}_{half}_{j}")
                    yps.append(yp)
                    for it in range(KT):
                        nc.tensor.matmul(
                            yp[:], xT_tiles[s][:, it, :],
                            wmixT[:, it, ocx * 512:(ocx + 1) * 512],
                            start=(it == 0), stop=(it == KT - 1))
                evict(s, half, yps, fine=fine)

            # first 8 s-tiles' half-0 passes (W chunks 0-3 only) chase the
            # W stream: ~58us of PE work before any half-1 pass needs W4-7.
            # The bf16 W4-7 upcasts interleave with the eviction stream.
            main_half(0, 0)
            main_half(1, 0)
            upcast_w(w4)
            w5 = fetch_w_bf16(5)
            main_half(2, 0)
            upcast_w(w5)
            w6 = fetch_w_bf16(6)
            main_half(3, 0)
            upcast_w(w6)
            w7 = fetch_w_bf16(7)
            main_half(4, 0)
            upcast_w(w7)
            for s in range(5, 8):
                main_half(s, 0)
            for s in range(8):
                main_half(s, 1)
                fetch_xT(8 + s)
            for s in range(8, ST):
                fine = s == ST - 1
                main_half(s, 0, fine=fine)
                fetch_xT(s + 7)
                main_half(s, 1, fine=fine)

    nc.compile()
    _BUILT[main_dt_name] = nc
    return nc


def _shard_layouts(inputs, main_dt_name=MAIN_DT_NAME):
    """Host-side shard/layout prep: fold the mixing into W/b, k-major tiles."""
    if main_dt_name == "bf16":
        import ml_dtypes
        io_np = ml_dtypes.bfloat16
    else:
        io_np = np.float32
    x = np.asarray(inputs["x"], np.float32)
    weights = np.asarray(inputs["weights"], np.float32)
    W = np.asarray(inputs["W"], np.float32)
    bias = np.asarray(inputs["b"], np.float32)

    W_mix, b_mix = _mix_np(weights, W, bias)
    bmix_rep = np.ascontiguousarray(
        np.broadcast_to(b_mix[None, :], (P, OUT)), dtype=np.float32)

    import ml_dtypes
    # WT[p, it, o] = W_mix[o, it*128+p], shipped bf16
    WT = np.ascontiguousarray(
        W_mix.reshape(OUT, KT, P).transpose(2, 1, 0).astype(ml_dtypes.bfloat16))
    shared = {"WT": WT, "bmix": bmix_rep}
    in_maps = []
    for c in range(N_CORES):
        # xT[s, p, it, q] = x[c, s*128+q, it*128+p]
        xT = np.ascontiguousarray(
            x[c].reshape(ST, P, KT, P).transpose(0, 3, 2, 1).astype(io_np))
        in_maps.append(dict(shared, xT=xT))
    return in_maps


def _run(inputs, main_dt_name=MAIN_DT_NAME, trace=False, tmpdir=None):
    _ensure_ntff_hook()
    import concourse.bass_utils as bass_utils
    # artifact upload needs a bucket; keep traces local
    bass_utils.upload_artifacts = lambda tmpdir: f"local:{tmpdir}"
    from concourse.bass_utils import run_bass_kernel_spmd

    nc = _build(main_dt_name)
    in_maps = _shard_layouts(inputs, main_dt_name)
    res = run_bass_kernel_spmd(nc, in_maps, core_ids=list(range(N_CORES)),
                               trace=trace, tmpdir=tmpdir)
    y = np.empty((B, S, OUT), np.float32)
    for c in range(N_CORES):
        y[c] = res.results[c]["y"]
    return y, res


def kernel(**inputs) -> np.ndarray:
    y, _ = _run(inputs, trace=False)
    return y


# revision 52
# speedup vs baseline: 1.0625x; 1.0625x over previous
"""Trainium2 Bass kernel for nn_MixedLinearV2 (moe_routing).

y[b,s,o] = sum_i x[b,s,i] * (W[o,i]*coeff[o,i]) + b[o]*rowscale[o]
  coeff[o,i]  = sum_k weights[k] * row_mask[k,o] * col_mask[k,i]
  rowscale[o] = sum_k weights[k] * row_mask[k,o]

Strategy: data-parallel over batch (8 batch elements -> 8 NeuronCores).
coeff/rowscale depend only on the 9 mixing weights and the static
masks, so W_mix = W*coeff and b_mix = b*rowscale are folded on the host
during the (already required) shard/layout prep; each core then runs a
pure [4096,1024]x[1024,4096] GEMM with a bias-add on eviction.

Per core: x and W_mix are laid out contraction-dim-major (i on
partitions) so they DMA straight into matmul operand tiles. W_mix ships
bf16 (8MB instead of 16MB -- the startup is DMA-bound) and is upcast to
f32r on the otherwise-idle DVE before the matmuls read it. Main loop:
for each 128-row tile of x: 64 matmuls (8 k-tiles x 8 512-wide out
chunks, 4 PSUM banks active + 4 draining), DVE bias-add eviction, DMA
out. The first 8 s-tiles' half-0 passes (chunks 0-3 only) chase the W
DMA stream so the PE never idles while W arrives; the last s-tile
evicts per-chunk so the final writeback drains early.

Matmul dtype: float32r for both operands (1 col/cycle on the PE --
227ns/matmul sustained, the same rate as bf16 but with ~13-bit
mantissas). Net norm relative error ~1.4e-3, dominated by the bf16
quantization of W_mix.
"""

import sys
import types

import numpy as np

# ---- constants (hardcoded from the problem spec) ----
B, S, IN, OUT = 8, 4096, 1024, 4096
IN_DIMS = (512, 768, 1024)
OUT_MULTS = (2, 3, 4)
K9 = 9
P = 128
KT = IN // P          # 8 k-tiles
ST = S // P           # 32 s-tiles
OC = OUT // 512       # 8 out chunks of 512
N_CORES = 8

MAIN_DT_NAME = "f32r"  # one of: f32r, bf16, f32


def _ensure_ntff_hook():
    """Register the antenv.axon_hooks shim so trace=True can profile."""
    if 'antenv.axon_hooks' in sys.modules:
        return
    try:
        import antenv
    except ImportError:
        return
    mod = types.ModuleType('antenv.axon_hooks')
    mod._hook = None
    mod.set_axon_ntff_profile_hook = lambda h: setattr(mod, '_hook', h)
    mod.get_axon_ntff_profile_hook = lambda: mod._hook
    sys.modules['antenv.axon_hooks'] = mod
    antenv.axon_hooks = mod
    try:
        from trn_agent_boot.trn_boot import _ntff_profile_via_ctypes
        mod.set_axon_ntff_profile_hook(
            _ntff_profile_via_ctypes('/opt/axon/libaxon_pjrt.so'))
    except Exception:
        pass


def _mix_np(weights, W, bias):
    """Host-side mixing: W_mix = W*coeff, b_mix = b*rowscale."""
    out_dims = np.array([m * i for i in IN_DIMS for m in OUT_MULTS])
    in_dims = np.array([i for i in IN_DIMS for _ in OUT_MULTS])
    row_mask = (np.arange(OUT)[None, :] < out_dims[:, None]).astype(np.float32)
    col_mask = (np.arange(IN)[None, :] < in_dims[:, None]).astype(np.float32)
    coeff = np.einsum('k,ko,ki->oi', weights, row_mask, col_mask,
                      dtype=np.float32)
    W_mix = (W * coeff).astype(np.float32)
    b_mix = (bias * (weights @ row_mask)).astype(np.float32)
    return W_mix, b_mix


_BUILT = {}


def _build(main_dt_name=MAIN_DT_NAME):
    """Build + compile the SPMD Bass program (one program, 8 cores)."""
    if main_dt_name in _BUILT:
        return _BUILT[main_dt_name]

    import concourse.bacc as bacc
    import concourse.mybir as mybir
    from concourse.tile import TileContext

    F32 = mybir.dt.float32
    F32R = mybir.dt.float32r
    MAIN_DT = {"f32r": F32R, "bf16": mybir.dt.bfloat16, "f32": F32}[main_dt_name]
    # x/W stream in pre-cast to MAIN_DT (sync DMA cannot cast)
    IO_DT = MAIN_DT

    nc = bacc.Bacc("TRN2", target_bir_lowering=False, debug=False,
                   num_devices=N_CORES)

    BF16 = mybir.dt.bfloat16
    # xT[s, p, it, q] = x[s*128+q, it*128+p] : k-major 128-row tiles
    xT_d = nc.declare_dram_parameter("xT", [ST, P, KT, P], IO_DT, isOutput=False)
    # WT[p, it, o] = W_mix[o, it*128+p], shipped bf16 (half the startup
    # DMA bytes) and DVE-upcast to MAIN_DT before the matmuls read it
    wT_d = nc.declare_dram_parameter("WT", [P, KT, OUT], BF16, isOutput=False)
    # b_mix replicated across partitions on host
    b_d = nc.declare_dram_parameter("bmix", [P, OUT], F32, isOutput=False)
    y_d = nc.declare_dram_parameter("y", [S, OUT], F32, isOutput=True)

    with TileContext(nc) as tc:
        with (
            tc.tile_pool(name="persist", bufs=1) as persist,
            tc.tile_pool(name="xT_pool", bufs=8) as xT_pool,
            tc.tile_pool(name="ysb_pool", bufs=2) as ysb_pool,
            tc.tile_pool(name="wstg_pool", bufs=3) as wstg_pool,
            tc.tile_pool(name="ps_pool", bufs=8, space="PSUM") as ps_pool,
        ):
            wmixT = persist.tile([P, KT, OUT], MAIN_DT)   # [i_part, i_outer, o]
            bias_sb = persist.tile([P, OUT], F32)

            # PE warm-up: dummy N=512 matmuls (100% streaming duty) fill
            # the DMA-wait window at kernel start so the HAM clock gate
            # flips to 8/8 (2.4GHz) before the real matmul stream begins
            # (~3.4us of busy PE flips it).
            warm_rhs = persist.tile([P, 512], MAIN_DT)
            nc.any.memzero(warm_rhs[:])
            warm_ps = ps_pool.tile([P, 512], F32, tag="ps", name="warm_ps")
            for _ in range(18):
                nc.tensor.matmul(warm_ps[:], warm_rhs[:, 0:P], warm_rhs[:],
                                 start=True, stop=True)

            xT_tiles = {}

            def fetch_xT(s):
                if s < ST and s not in xT_tiles:
                    xT_tiles[s] = xT_pool.tile([P, KT, P], MAIN_DT, tag="xT",
                                               name=f"xT_{s}")
                    nc.sync.dma_start(xT_tiles[s][:], xT_d[s])

            def fetch_w_bf16(ocx):
                """Dispatch the bf16 DMAs for a W chunk (two half-chunks)."""
                tiles = []
                for h in range(2):
                    lo = ocx * 512 + h * 256
                    wstg = wstg_pool.tile([P, KT, 256], BF16, tag="wstg",
                                          name=f"wstg_{ocx}_{h}")
                    nc.sync.dma_start(wstg[:], wT_d[:, :, lo:lo + 256])
                    tiles.append((lo, wstg))
                return tiles

            def upcast_w(tiles, fine=False):
                if fine:
                    # per-k-slice copies, both halves interleaved, so the
                    # first matmul group waits on 2 copies instead of 9
                    for it in range(KT):
                        for lo, wstg in tiles:
                            nc.vector.tensor_copy(
                                wmixT[:, it, lo:lo + 256], wstg[:, it, :])
                else:
                    for lo, wstg in tiles:
                        nc.vector.tensor_copy(wmixT[:, :, lo:lo + 256],
                                              wstg[:])

            # DMA FIFO: x0 first, W chunks right behind it (all bf16,
            # upcast on the DVE), x tiles and bias interleaved by the time
            # each is first needed.
            fetch_xT(0)
            w0 = fetch_w_bf16(0)
            w1 = fetch_w_bf16(1)
            upcast_w(w0, fine=True)
            upcast_w(w1, fine=True)
            fetch_xT(1)
            nc.sync.dma_start(bias_sb[:, :OUT // 2], b_d[:, :OUT // 2])
            w2 = fetch_w_bf16(2)
            w3 = fetch_w_bf16(3)
            upcast_w(w2)
            upcast_w(w3)
            fetch_xT(2)
            fetch_xT(3)
            nc.sync.dma_start(bias_sb[:, OUT // 2:], b_d[:, OUT // 2:])
            fetch_xT(4)
            w4 = fetch_w_bf16(4)
            fetch_xT(5)
            fetch_xT(6)
            fetch_xT(7)

            def evict(s, half, yps, fine=False):
                ysb = ysb_pool.tile([P, 2048], F32, tag="ysb",
                                    name=f"ysb_{s}_{half}")
                for j in range(4):
                    ocx = half * 4 + j
                    nc.vector.tensor_tensor(
                        ysb[:, j * 512:(j + 1) * 512], yps[j][:],
                        bias_sb[:, ocx * 512:(ocx + 1) * 512],
                        mybir.AluOpType.add)
                    if fine:
                        # per-chunk DMA so the final writeback pipelines
                        # behind each bank instead of waiting for all four
                        nc.sync.dma_start(
                            y_d[s * P:(s + 1) * P,
                                ocx * 512:(ocx + 1) * 512],
                            ysb[:, j * 512:(j + 1) * 512])
                if not fine:
                    nc.sync.dma_start(
                        y_d[s * P:(s + 1) * P,
                            half * 2048:(half + 1) * 2048],
                        ysb[:])

            def main_half(s, half, fine=False):
                yps = []
                for j in range(4):
                    ocx = half * 4 + j
                    yp = ps_pool.tile([P, 512], F32, tag="ps",
                                      name=f"yps_{s}_{half}_{j}")
                    yps.append(yp)
                    for it in range(KT):
                        nc.tensor.matmul(
                            yp[:], xT_tiles[s][:, it, :],
                            wmixT[:, it, ocx * 512:(ocx + 1) * 512],
                            start=(it == 0), stop=(it == KT - 1))
                evict(s, half, yps, fine=fine)

            # first 8 s-tiles' half-0 passes (W chunks 0-3 only) chase the
            # W stream: ~58us of PE work before any half-1 pass needs W4-7.
            # The bf16 W4-7 upcasts interleave with the eviction stream.
            main_half(0, 0)
            main_half(1, 0)
            upcast_w(w4)
            w5 = fetch_w_bf16(5)
            main_half(2, 0)
            upcast_w(w5)
            w6 = fetch_w_bf16(6)
            main_half(3, 0)
            upcast_w(w6)
            w7 = fetch_w_bf16(7)
            main_half(4, 0)
            upcast_w(w7)
            for s in range(5, 8):
                main_half(s, 0)
            for s in range(8):
                main_half(s, 1)
                fetch_xT(8 + s)
            for s in range(8, ST):
                fine = s == ST - 1
                main_half(s, 0, fine=fine)
                fetch_xT(s + 7)
                main_half(s, 1, fine=fine)

    nc.compile()
    _BUILT[main_dt_name] = nc
    return nc


def _shard_layouts(inputs, main_dt_name=MAIN_DT_NAME):
    """Host-side shard/layout prep: fold the mixing into W/b, k-major tiles."""
    if main_dt_name == "bf16":
        import ml_dtypes
        io_np = ml_dtypes.bfloat16
    else:
        io_np = np.float32
    x = np.asarray(inputs["x"], np.float32)
    weights = np.asarray(inputs["weights"], np.float32)
    W = np.asarray(inputs["W"], np.float32)
    bias = np.asarray(inputs["b"], np.float32)

    W_mix, b_mix = _mix_np(weights, W, bias)
    bmix_rep = np.ascontiguousarray(
        np.broadcast_to(b_mix[None, :], (P, OUT)), dtype=np.float32)

    import ml_dtypes
    # WT[p, it, o] = W_mix[o, it*128+p], shipped bf16
    WT = np.ascontiguousarray(
        W_mix.reshape(OUT, KT, P).transpose(2, 1, 0).astype(ml_dtypes.bfloat16))
    shared = {"WT": WT, "bmix": bmix_rep}
    in_maps = []
    for c in range(N_CORES):
        # xT[s, p, it, q] = x[c, s*128+q, it*128+p]
        xT = np.ascontiguousarray(
            x[c].reshape(ST, P, KT, P).transpose(0, 3, 2, 1).astype(io_np))
        in_maps.append(dict(shared, xT=xT))
    return in_maps


def _run(inputs, main_dt_name=MAIN_DT_NAME, trace=False, tmpdir=None):
    _ensure_ntff_hook()
    import concourse.bass_utils as bass_utils
    # artifact upload needs a bucket; keep traces local
    bass_utils.upload_artifacts = lambda tmpdir: f"local:{tmpdir}"
    from concourse.bass_utils import run_bass_kernel_spmd

    nc = _build(main_dt_name)
    in_maps = _shard_layouts(inputs, main_dt_name)
    res = run_bass_kernel_spmd(nc, in_maps, core_ids=list(range(N_CORES)),
                               trace=trace, tmpdir=tmpdir)
    y = np.empty((B, S, OUT), np.float32)
    for c in range(N_CORES):
        y[c] = res.results[c]["y"]
    return y, res


def kernel(**inputs) -> np.ndarray:
    y, _ = _run(inputs, trace=False)
    return y
